# revision 5
# baseline (speedup 1.0000x reference)
"""GCNConv (PyG, bias=False) on 8 Trainium2 NeuronCores.

out = D^{-1/2} (A+I) D^{-1/2} (x @ W)

Strategy: since the op is linear, aggregate first, project second:
  z = dis * x                              (host; dis = rsqrt(degree))
  agg[d] = sum_{src->d (incl self)} z[src] (device: dma_gather + one-hot matmul)
  out[d] = dis[d] * agg[d] @ W             (device: dis folded into the one-hot)

Node rows (outputs) are partitioned across the 8 cores; each core's edges are
sorted by (128-dst window, 25000-row src chunk).  For each 128-edge tile the
device gathers z rows by source index (dma_gather, int16 local indices),
builds S[e,d] = (dstoff_e == d) * dis_dst_e with one DVE tensor_scalar from a
constant iota matrix, and accumulates aggT[feat,dst] += slab_tile^T @ S in
PSUM over the window.  aggT then feeds matmul(lhsT=aggT, rhs=W) directly
(contraction dim = feat is already on partitions), so no transposes are
needed anywhere.  The schedule (tiles per window/chunk group) is the
max across cores, so all 8 cores share one NEFF; padding lanes carry
dis_dst = 0 and gather row 0, contributing exactly zero.
"""
import sys

sys.path.insert(0, '/opt/trn_rl_repo')

import numpy as np

N_NODES = 100000
N_EDGES = 1600000
DIM = 128
N_CORES = 8
NPC = N_NODES // N_CORES          # dst rows per core (12500)
WIN = 128                         # dsts per window
NW = (NPC + WIN - 1) // WIN       # windows per core (98; last window 84 dsts)
CHUNK = 25000                     # src rows per gather-table chunk (int16 limit)
NQ = (N_NODES + CHUNK - 1) // CHUNK
TILE = 128                        # edges per tile
MAX_CALL_TILES = 8                # 1024 idxs per dma_gather (64-desc/engine cap)

_patched = False


def _setup_concourse():
    global _patched
    if _patched:
        return
    _patched = True
    import bass_rust
    import concourse.bass as bass
    import concourse.tile as tile

    # Walrus in this container allows exactly ONE sync-wait per instruction.
    # (1) Tile's end-of-context drain can carry several: split extra waits
    # onto chained Drain instructions.
    def _patched_drain_and_barrier(self, tick_clock, wait_clock):
        from concourse.vector_clock import ScopedClock
        nc = self.nc
        drain_inst = nc.sync.drain()
        wait_clock.add_sem_waits(drain_inst.ins,
                                 ScopedClock({None: tick_clock.global_clock}))
        si = drain_inst.ins.sync_info
        waits = list(si.on_wait or []) if si is not None else []
        if len(waits) > 1:
            si.on_wait = waits[:1]
            for w in waits[1:]:
                d2 = nc.sync.drain()
                d2.ins.sync_info = bass_rust.SyncInfo(on_wait=[w], on_update=[])
        nc.all_engine_barrier()
        popped = nc._tile_sem_poison_stack.pop()
        assert popped is self._sem_poison
        nc.clear_and_free_semaphores(list(self.sems.allocated().values()))
        nc.all_engine_barrier()

    tile.TileContext._drain_and_barrier = _patched_drain_and_barrier

    # (2) Any other instruction with >1 waits: move extras onto NoOp
    # carriers on the same engine immediately before it.
    def _legalize_waits(m):
        for f in m.functions:
            for blk in f.blocks:
                insts = blk.instructions
                out = []
                changed = False
                for inst in insts:
                    si = inst.sync_info
                    waits = list(si.on_wait or []) if si is not None else []
                    if len(waits) > 1:
                        changed = True
                        for k, w in enumerate(waits[:-1]):
                            nop = bass_rust.InstNoOp(
                                name=f"{inst.name}-wsplit{k}", ins=[], outs=[])
                            nop.engine = inst.engine
                            nop.sync_info = bass_rust.SyncInfo(
                                on_wait=[w], on_update=[])
                            out.append(nop)
                        si.on_wait = waits[-1:]
                    out.append(inst)
                if changed:
                    blk.instructions = out

    orig_to_json_bytes = bass.Bass.to_json_bytes
    if not getattr(bass.Bass, "_wsplit_patch", False):
        def _patched_to_json_bytes(self):
            _legalize_waits(self.m)
            return orig_to_json_bytes(self)
        bass.Bass.to_json_bytes = _patched_to_json_bytes
        bass.Bass._wsplit_patch = True


def _preprocess(x, edge_index, W):
    """Host-side sharding: per-core padded edge arrays + shared schedule."""
    x = np.asarray(x, dtype=np.float32)
    W = np.asarray(W, dtype=np.float32)
    ei = np.asarray(edge_index)
    src = np.concatenate([ei[0], np.arange(N_NODES, dtype=ei.dtype)]).astype(np.int64)
    dst = np.concatenate([ei[1], np.arange(N_NODES, dtype=ei.dtype)]).astype(np.int64)

    deg = np.bincount(dst, minlength=N_NODES).astype(np.float32)
    dis = 1.0 / np.sqrt(np.maximum(deg, 1.0))
    z = x * dis[:, None]                                 # gather table

    core = dst // NPC
    dloc = dst - core * NPC
    w = dloc // WIN
    dstoff = (dloc - w * WIN).astype(np.float32)
    q = src // CHUNK
    srcloc = (src - q * CHUNK).astype(np.int16)
    dd = dis[dst]                                        # dis of each edge's dst

    key = (core * NW + w) * NQ + q
    order = np.argsort(key, kind='stable')
    key_s = key[order]
    cnt = np.bincount(key, minlength=N_CORES * NW * NQ).reshape(N_CORES, NW, NQ)
    # shared schedule: tiles per (window, chunk) group = max over cores
    T = (-(-cnt // TILE)).max(axis=0)                    # [NW, NQ] int
    group_off = np.concatenate([[0], np.cumsum(T.reshape(-1) * TILE)])
    L = int(group_off[-1])                               # padded edges per core

    # position of each (sorted) edge inside its core's padded layout
    first_idx = np.searchsorted(key_s, np.arange(N_CORES * NW * NQ), side='left')
    rank = np.arange(key_s.size) - first_idx[key_s]      # rank within group
    pos = group_off[key_s % (NW * NQ)] + rank            # [sorted edges]

    idx_arrs, dst_arrs, dd_arrs = [], [], []
    srcloc_s = srcloc[order]
    dstoff_s = dstoff[order]
    dd_s = dd[order].astype(np.float32)
    core_s = key_s // (NW * NQ)
    for c in range(N_CORES):
        m = core_s == c
        ia = np.zeros(L, np.int16)          # pad: gather row 0 (valid, inert)
        da = np.zeros(L, np.float32)
        dda = np.zeros(L, np.float32)       # pad: dis_dst = 0 -> contributes 0
        p = pos[m]
        ia[p] = srcloc_s[m]
        da[p] = dstoff_s[m]
        dda[p] = dd_s[m]
        # dma_gather index layout: idx i -> [i % 16, i // 16], x8 replicated
        idx_arrs.append(np.ascontiguousarray(
            np.tile(ia.reshape(-1, 16).T, (8, 1))))
        dst_arrs.append(np.ascontiguousarray(da.reshape(-1, TILE).T))
        dd_arrs.append(np.ascontiguousarray(dda.reshape(-1, TILE).T))

    iota = np.ascontiguousarray(
        np.tile(np.arange(WIN, dtype=np.float32), (TILE, 1)))
    return z, W, T, idx_arrs, dst_arrs, dd_arrs, iota


def _build(T):
    """Build the shared SPMD bass program from the schedule T [NW, NQ]."""
    import concourse.bacc as bacc
    import concourse.mybir as mybir
    import concourse.tile as tile

    tot_tiles = int(T.sum())
    L = tot_tiles * TILE

    nc = bacc.Bacc("TRN2", target_bir_lowering=False, debug=False)
    z_ds = [nc.dram_tensor(f"z{q}", [min(CHUNK, N_NODES - q * CHUNK), DIM],
                           mybir.dt.float32, kind="ExternalInput")
            for q in range(NQ)]
    idx_d = nc.dram_tensor("idxs", [128, L // 16], mybir.dt.int16, kind="ExternalInput")
    dst_d = nc.dram_tensor("dstv", [128, tot_tiles], mybir.dt.float32, kind="ExternalInput")
    dd_d = nc.dram_tensor("ddv", [128, tot_tiles], mybir.dt.float32, kind="ExternalInput")
    iota_d = nc.dram_tensor("iota", [128, WIN], mybir.dt.float32, kind="ExternalInput")
    W_d = nc.dram_tensor("W", [DIM, DIM], mybir.dt.float32, kind="ExternalInput")
    out_d = nc.dram_tensor("out", [NPC, DIM], mybir.dt.float32, kind="ExternalOutput")

    with tile.TileContext(nc) as tc:
        with tc.tile_pool(name="const", bufs=1) as cpool, \
             tc.tile_pool(name="gather", bufs=6) as gpool, \
             tc.tile_pool(name="sel", bufs=6) as spool, \
             tc.tile_pool(name="stage", bufs=3) as apool, \
             tc.tile_pool(name="pagg", bufs=3, space="PSUM") as pagg, \
             tc.tile_pool(name="pout", bufs=2, space="PSUM") as pout:

            idxs = cpool.tile([128, L // 16], mybir.dt.int16)
            nc.sync.dma_start(out=idxs[:], in_=idx_d[:])
            dstv = cpool.tile([128, tot_tiles], mybir.dt.float32)
            nc.sync.dma_start(out=dstv[:], in_=dst_d[:])
            ddv = cpool.tile([128, tot_tiles], mybir.dt.float32)
            nc.sync.dma_start(out=ddv[:], in_=dd_d[:])
            iota = cpool.tile([128, WIN], mybir.dt.float32)
            nc.sync.dma_start(out=iota[:], in_=iota_d[:])
            Wt = cpool.tile([DIM, DIM], mybir.dt.float32)
            nc.sync.dma_start(out=Wt[:], in_=W_d[:])

            gt = 0    # global tile counter
            import os as _os
            _maxw = int(_os.environ.get("K_MAX_WINDOWS", NW))
            for w in range(min(NW, _maxw)):
                wlen = min(WIN, NPC - w * WIN)
                tiles_w = int(T[w].sum())
                psum_agg = pagg.tile([128, WIN], mybir.dt.float32)
                ti = 0
                for q in range(NQ):
                    tq = int(T[w, q])
                    zq = z_ds[q][:]
                    c0 = 0
                    while c0 < tq:
                        nt = min(MAX_CALL_TILES, tq - c0)
                        slab = gpool.tile([128, MAX_CALL_TILES, DIM],
                                          mybir.dt.float32, tag="slab")
                        n_idx = nt * TILE
                        nc.gpsimd.dma_gather(
                            slab[:, :nt, :], zq,
                            idxs[:, (gt * TILE) // 16:(gt * TILE + n_idx) // 16],
                            n_idx, n_idx, DIM)
                        for t in range(nt):
                            S = spool.tile([TILE, WIN], mybir.dt.float32, tag="S")
                            nc.vector.tensor_scalar(
                                out=S[:], in0=iota[:],
                                scalar1=dstv[:, gt + t:gt + t + 1],
                                scalar2=ddv[:, gt + t:gt + t + 1],
                                op0=mybir.AluOpType.is_equal,
                                op1=mybir.AluOpType.mult)
                            nc.tensor.matmul(
                                out=psum_agg[:], lhsT=slab[:, t, :], rhs=S[:],
                                start=(ti == 0), stop=(ti == tiles_w - 1))
                            ti += 1
                        gt += nt
                        c0 += nt
                aggT = apool.tile([128, WIN], mybir.dt.float32, tag="aggT")
                nc.vector.tensor_copy(out=aggT[:], in_=psum_agg[:])
                psum_o = pout.tile([WIN, DIM], mybir.dt.float32)
                nc.tensor.matmul(out=psum_o[:], lhsT=aggT[:], rhs=Wt[:],
                                 start=True, stop=True)
                osb = apool.tile([WIN, DIM], mybir.dt.float32, tag="osb")
                nc.vector.tensor_copy(out=osb[:], in_=psum_o[:])
                nc.sync.dma_start(out=out_d[w * WIN:w * WIN + wlen, :],
                                  in_=osb[:wlen, :])
    nc.compile()
    return nc


def kernel(x, edge_index, W):
    _setup_concourse()
    from concourse.bass_utils import run_bass_kernel_spmd

    z, W32, T, idx_arrs, dst_arrs, dd_arrs, iota = _preprocess(x, edge_index, W)
    nc = _build(T)

    in_maps = []
    for c in range(N_CORES):
        im = {"idxs": idx_arrs[c], "dstv": dst_arrs[c],
              "ddv": dd_arrs[c], "iota": iota, "W": W32}
        for q in range(NQ):
            im[f"z{q}"] = np.ascontiguousarray(z[q * CHUNK:(q + 1) * CHUNK])
        in_maps.append(im)
    res = run_bass_kernel_spmd(nc, in_maps, core_ids=list(range(N_CORES)))
    out = np.empty((N_NODES, DIM), np.float32)
    for c in range(N_CORES):
        out[c * NPC:(c + 1) * NPC] = res.results[c]["out"]
    return out
